# revision 1
# baseline (speedup 1.0000x reference)
"""Multi-head attention (B=2, S=2048, DM=1024, H=16, DH=64, causal) on 8 TRN2 cores.

Sharding: tensor-parallel over heads. Core c owns heads {2c, 2c+1} = q/k/v dims
[128c, 128c+128). Each core computes its QKV projections, causal attention for
its 2 heads (both batches), and a partial output projection (row-parallel over
Wo). Host unshards by summing the 8 partials and adding bo (the TP all-reduce).

In-kernel layout (per core), all matmuls bf16 with fp32 PSUM accumulation:
  - xT (1024 feat, 4096 seq=b*2048+s) bf16, replicated across cores.
  - QT/KT: (128 qdim [h0 d0-63 | h1 d0-63], 4096 seq) bf16 in SBUF.
  - V: natural layout (128 seq-keys, 128 vdim) tiles in SBUF.
  - scoresT tiles (keys on partitions, queries free): 2 heads packed on PE
    row-groups (K=64 each, concurrent); h0 -> PSUM bank 0, h1 -> bank 1
    (concurrent matmuls into the same PSUM bank fault the core).
  - exp on ScalarE over both banks in one instruction.
  - AV: col-packed 2 heads -> ctxT (128 [h0 d | h1 d], 512q) PSUM accumulation.
  - softmax denominators: ones-vector matmuls col-packed at partitions 0/32.
  - normalization + output projection DEFERRED to a tail phase (unnormalized
    ctx and sums stashed in SBUF per q-block) so the long reciprocal/broadcast
    chain never blocks the attention pipeline.

Causality is hardcoded (the reference's attention_mask is always triu causal):
above-diagonal tiles are skipped entirely, diagonal tiles get a triangular
bf16 multiplicative mask generated in-kernel.
"""

import os
import sys

import numpy as np

try:
    import concourse  # noqa: F401
except ImportError:
    sys.path.insert(0, "/opt/trn_rl_repo")

import ml_dtypes

BF16 = ml_dtypes.bfloat16

B, S, DM = 2, 2048, 1024
H, DH = 16, 64
NCORES = 8
CPC = DM // NCORES  # 128 q/k/v dims per core (2 heads)
BS = B * S  # 4096
Q_W = 512  # query-block width

_CACHE = {}
LAST_EXEC_NS = None
LAST_RESULTS = None


def _build(repeat=1):
    # dev-only ablation switches for benchmarking (unset in grading)
    skip = set(os.environ.get("BENCH_SKIP", "").split(",")) - {""}
    import concourse.mybir as mybir
    from concourse import bacc
    from concourse import tile
    from concourse.masks import make_identity, make_upper_triangular

    f32 = mybir.dt.float32
    f16 = mybir.dt.float16
    bf16 = mybir.dt.bfloat16
    Exp = mybir.ActivationFunctionType.Exp
    Ident = mybir.ActivationFunctionType.Identity

    nc = bacc.Bacc(
        "TRN2",
        target_bir_lowering=False,
        debug=False,
        enable_asserts=False,
        num_devices=NCORES,
    )

    xT = nc.dram_tensor("xT", (DM, BS), bf16, kind="ExternalInput").ap()
    wq = nc.dram_tensor("wq", (DM, CPC), bf16, kind="ExternalInput").ap()
    wk = nc.dram_tensor("wk", (DM, CPC), bf16, kind="ExternalInput").ap()
    wv = nc.dram_tensor("wv", (DM, CPC), bf16, kind="ExternalInput").ap()
    wo = nc.dram_tensor("wo", (CPC, DM), bf16, kind="ExternalInput").ap()
    bq = nc.dram_tensor("bq", (CPC, 1), f32, kind="ExternalInput").ap()
    bk = nc.dram_tensor("bk", (CPC, 1), f32, kind="ExternalInput").ap()
    bv = nc.dram_tensor("bv", (CPC, 1), f32, kind="ExternalInput").ap()
    out = nc.dram_tensor("out", (BS, DM), f16, kind="ExternalOutput").ap()

    KT_FEAT = DM // 128  # 8 contraction tiles

    with tile.TileContext(nc) as tc:
      with tc.tile_pool(name="consts", bufs=1) as consts, \
           tc.tile_pool(name="sb", bufs=2) as sb, \
           tc.tile_pool(name="psp", bufs=1, space="PSUM") as psp:

        def body():
            # ---- persistent tiles ------------------------------------------
            wq_sb = consts.tile((128, KT_FEAT, CPC), bf16, name="wq_sb")
            wk_sb = consts.tile((128, KT_FEAT, CPC), bf16, name="wk_sb")
            wv_sb = consts.tile((128, KT_FEAT, CPC), bf16, name="wv_sb")
            wo_sb = consts.tile((CPC, DM), bf16, name="wo_sb")
            bq_sb = consts.tile((CPC, 1), f32, name="bq_sb")
            bk_sb = consts.tile((CPC, 1), f32, name="bk_sb")
            bv_sb = consts.tile((CPC, 1), f32, name="bv_sb")
            nc.sync.dma_start(wq_sb[:], wq.rearrange("(t p) m -> p t m", p=128))
            nc.sync.dma_start(wk_sb[:], wk.rearrange("(t p) m -> p t m", p=128))
            nc.sync.dma_start(wv_sb[:], wv.rearrange("(t p) m -> p t m", p=128))
            nc.sync.dma_start(wo_sb[:], wo)
            nc.sync.dma_start(bq_sb[:], bq)
            nc.sync.dma_start(bk_sb[:], bk)
            nc.sync.dma_start(bv_sb[:], bv)

            ones_col = consts.tile((128, 1), bf16, name="ones_col")
            nc.vector.memset(ones_col[:], 1.0)
            ones_f33 = consts.tile((33, 128), f32, name="ones_f33")
            nc.vector.memset(ones_f33[:], 1.0)
            ident_sb = consts.tile((128, 128), bf16, name="ident_sb")
            make_identity(nc, ident_sb[:])
            # causal mask tile: mask[k, q] = 1.0 if k <= q else 0.0
            cmask = consts.tile((128, 128), bf16, name="cmask")
            make_upper_triangular(nc, cmask[:], val=1.0, diag=True)

            QT_sb = consts.tile((128, BS), bf16, name="QT_sb")
            KT_sb = consts.tile((128, BS), bf16, name="KT_sb")
            V_sb = consts.tile((128, BS // 128, 128), bf16, name="V_sb")

            xT3 = xT.rearrange("(t p) q -> p t q", p=128)
            xT_sb = consts.tile((128, KT_FEAT, BS), bf16, name="xT_sb")
            for t in range(KT_FEAT):
                nc.sync.dma_start(xT_sb[:, t:t + 1, :], xT3[:, t:t + 1, :])

            # ---- QKV projections -------------------------------------------
            if "qkv" in skip:
                nc.vector.memset(QT_sb[:], 0.01)
                nc.vector.memset(KT_sb[:], 0.01)
                nc.vector.memset(V_sb[:], 0.01)
            N_CH = BS // 512
            for ch in range(0 if "qkv" not in skip else N_CH, N_CH):
                c0 = ch * 512

                vt_sb = sb.tile((128, 512), bf16, name=f"vt{ch}",
                                tag="vt", bufs=2)
                for pname, w_sb, b_sb, out_T in (
                    ("q", wq_sb, bq_sb, QT_sb),
                    ("k", wk_sb, bk_sb, KT_sb),
                    ("v", wv_sb, bv_sb, vt_sb),
                ):
                    ps_p = psp.tile((128, 1024), f32, name=f"ps_{pname}{ch}",
                                    tag="big", bufs=3)
                    for t in range(KT_FEAT):
                        nc.tensor.matmul(
                            ps_p[:, :512],
                            lhsT=w_sb[:, t, :],
                            rhs=xT_sb[:, t, c0:c0 + 512],
                            start=(t == 0),
                            stop=(t == KT_FEAT - 1),
                        )
                    if pname == "v":
                        nc.scalar.activation(
                            out_T[:, :], ps_p[:, :512], Ident, bias=b_sb[:],
                        )
                    else:
                        nc.scalar.activation(
                            out_T[:, c0:c0 + 512], ps_p[:, :512], Ident,
                            bias=b_sb[:],
                        )

                # V natural layout via PE transpose of the VT chunk
                for sub in range(4):
                    ps_t = psp.tile((128, 128), bf16, name=f"ps_t{ch}_{sub}",
                                    tag="ctx", bufs=1)
                    nc.tensor.transpose(
                        ps_t[:],
                        vt_sb[:, sub * 128:(sub + 1) * 128],
                        ident_sb[:],
                    )
                    nc.vector.tensor_copy(V_sb[:, ch * 4 + sub, :], ps_t[:])

            # ---- attention (unnormalized) ----------------------------------
            NQB = S // Q_W  # q-blocks per batch
            ctxu = {}
            sums_sb = {}
            for b in range(B):
                for qb in range(NQB):
                    qb0 = qb * Q_W
                    g0 = b * S + qb0
                    n_t = (qb0 + Q_W) // 128  # causal: k-tiles needed

                    if "attn" in skip:
                        cu = consts.tile((128, Q_W), bf16, name=f"ctxu{b}_{qb}")
                        nc.vector.memset(cu[:], 0.01)
                        sm = consts.tile((33, Q_W), f32, name=f"sums{b}_{qb}")
                        nc.vector.memset(sm[:], 1.0)
                        ctxu[b, qb] = cu
                        sums_sb[b, qb] = sm
                        continue
                    ps_ctx = psp.tile((128, Q_W), f32, name=f"ps_ctx{b}_{qb}",
                                      tag="ctx", bufs=1)
                    ps_sums = psp.tile((33, Q_W), f32, name=f"ps_sums{b}_{qb}",
                                       tag="sums", bufs=1)

                    for t in range(n_t):
                        k0 = 128 * t
                        off = max(0, k0 - qb0)  # causal left-clip
                        w = Q_W - off
                        diag = k0 >= qb0
                        first = t == 0
                        last = t == n_t - 1
                        ps_s = psp.tile((128, 1024), f32,
                                        name=f"ps_s{b}_{qb}_{t}",
                                        tag="big", bufs=3)
                        exp_sb = sb.tile((128, 1024), bf16,
                                         name=f"exp{b}_{qb}_{t}",
                                         tag="exp", bufs=4)
                        for h in range(2):
                            nc.tensor.matmul(
                                ps_s[:, h * 512:h * 512 + w],
                                lhsT=KT_sb[h * 64:(h + 1) * 64,
                                           b * S + k0:b * S + k0 + 128],
                                rhs=QT_sb[h * 64:(h + 1) * 64,
                                          g0 + off:g0 + Q_W],
                                start=True,
                                stop=True,
                                tile_position=(h * 64, 0),
                                skip_group_check=True,
                            )
                        if off == 0:
                            nc.scalar.activation(
                                exp_sb[:, :1024], ps_s[:, :1024], Exp,
                                scale=0.125,
                            )
                        else:
                            nc.scalar.activation(
                                exp_sb[:, :w], ps_s[:, :w], Exp, scale=0.125,
                            )
                            nc.scalar.activation(
                                exp_sb[:, 512:512 + w], ps_s[:, 512:512 + w],
                                Exp, scale=0.125,
                            )
                        for h in range(2):
                            sb0 = h * 512
                            if diag:  # triangular mask on the diagonal block
                                nc.vector.tensor_mul(
                                    exp_sb[:, sb0:sb0 + 128],
                                    exp_sb[:, sb0:sb0 + 128],
                                    cmask[:],
                                )
                            if "av" in skip:
                                continue
                            nc.tensor.matmul(
                                ps_ctx[h * 64:(h + 1) * 64, off:Q_W],
                                lhsT=V_sb[:, (b * S + k0) // 128,
                                          h * 64:(h + 1) * 64],
                                rhs=exp_sb[:, sb0:sb0 + w],
                                start=first,
                                stop=last,
                                tile_position=(0, h * 64),
                                skip_group_check=True,
                            )
                            if "sums" in skip:
                                continue
                            nc.tensor.matmul(
                                ps_sums[h * 32:h * 32 + 1, off:Q_W],
                                lhsT=ones_col[:, :1],
                                rhs=exp_sb[:, sb0:sb0 + w],
                                start=first,
                                stop=last,
                                tile_position=(0, h * 32),
                                skip_group_check=True,
                            )

                    # stash unnormalized ctx + sums; frees PSUM fast so the
                    # next q-block is never blocked on the normalize chain
                    cu = consts.tile((128, Q_W), bf16, name=f"ctxu{b}_{qb}")
                    if "av" not in skip:
                        nc.vector.tensor_copy(cu[:], ps_ctx[:])
                    else:
                        nc.vector.tensor_copy(cu[:], exp_sb[:, :Q_W])
                    sm = consts.tile((33, Q_W), f32, name=f"sums{b}_{qb}")
                    if "sums" not in skip and "av" not in skip:
                        nc.vector.tensor_copy(sm[0:1, :], ps_sums[0:1, :])
                        nc.vector.tensor_copy(sm[32:33, :], ps_sums[32:33, :])
                    else:
                        nc.vector.memset(sm[:], 1.0)
                    ctxu[b, qb] = cu
                    sums_sb[b, qb] = sm

            # ---- tail: normalize + output projection -----------------------
            for b in range(B):
                for qb in range(NQB):
                    g0 = b * S + qb * Q_W
                    cu = ctxu[b, qb]
                    sm = sums_sb[b, qb]
                    rec = sb.tile((33, Q_W), f32, name=f"rec{b}_{qb}",
                                  tag="rec", bufs=2)
                    nc.vector.reciprocal(rec[0:1, :], sm[0:1, :])
                    nc.vector.reciprocal(rec[32:33, :], sm[32:33, :])
                    # broadcast reciprocal rows across partitions via K=1
                    # ones-matmuls (col-packed, disjoint partitions)
                    ps_bc = psp.tile((128, Q_W), f32, name=f"ps_bc{b}_{qb}",
                                     tag="big", bufs=3)
                    nc.tensor.matmul(
                        ps_bc[0:64, :], lhsT=ones_f33[0:1, 0:64],
                        rhs=rec[0:1, :],
                        start=True, stop=True, tile_position=(0, 0),
                        skip_group_check=True,
                    )
                    nc.tensor.matmul(
                        ps_bc[64:128, :], lhsT=ones_f33[32:33, 64:128],
                        rhs=rec[32:33, :],
                        start=True, stop=True, tile_position=(32, 64),
                        skip_group_check=True,
                    )
                    rec_bc = sb.tile((128, Q_W), f32, name=f"recbc{b}_{qb}",
                                     tag="recbc", bufs=2)
                    nc.vector.tensor_copy(rec_bc[:], ps_bc[:])
                    ctx_sb = sb.tile((128, Q_W), bf16, name=f"ctx{b}_{qb}",
                                     tag="ctx_sb", bufs=2)
                    nc.vector.tensor_mul(ctx_sb[:], cu[:], rec_bc[:])

                    for sub in range(Q_W // 128):
                        ps_o = psp.tile((128, 1024), f32,
                                        name=f"ps_o{b}_{qb}_{sub}",
                                        tag="big", bufs=3)
                        for nn in range(2):
                            nc.tensor.matmul(
                                ps_o[:, nn * 512:(nn + 1) * 512],
                                lhsT=ctx_sb[:, sub * 128:(sub + 1) * 128],
                                rhs=wo_sb[:, nn * 512:(nn + 1) * 512],
                                start=True,
                                stop=True,
                            )
                        o_sb = sb.tile((128, DM), f16,
                                       name=f"o_sb{b}_{qb}_{sub}",
                                       tag="o_sb", bufs=3)
                        nc.vector.tensor_copy(o_sb[:], ps_o[:, :DM])
                        r0 = g0 + sub * 128
                        nc.sync.dma_start(out[r0:r0 + 128, :], o_sb[:])

        if repeat == 1:
            body()
        else:
            with tc.For_i(0, repeat, 1):
                body()

    nc.compile()
    return nc


def _prep_inputs(x, Wq, bq, Wk, bk, Wv, bv, Wo):
    """Build the 8 per-core input maps (host-side sharding)."""
    x = np.asarray(x, dtype=np.float32)
    xT = np.ascontiguousarray(x.reshape(BS, DM).T).astype(BF16)
    in_maps = []
    for c in range(NCORES):
        sl = slice(c * CPC, (c + 1) * CPC)
        in_maps.append({
            "xT": xT,
            "wq": np.ascontiguousarray(np.asarray(Wq, np.float32)[sl, :].T).astype(BF16),
            "wk": np.ascontiguousarray(np.asarray(Wk, np.float32)[sl, :].T).astype(BF16),
            "wv": np.ascontiguousarray(np.asarray(Wv, np.float32)[sl, :].T).astype(BF16),
            "wo": np.ascontiguousarray(np.asarray(Wo, np.float32)[:, sl].T).astype(BF16),
            "bq": np.asarray(bq, np.float32)[sl].reshape(CPC, 1).copy(),
            "bk": np.asarray(bk, np.float32)[sl].reshape(CPC, 1).copy(),
            "bv": np.asarray(bv, np.float32)[sl].reshape(CPC, 1).copy(),
        })
    return in_maps


def _run(in_maps, trace=False):
    global LAST_EXEC_NS, LAST_RESULTS
    from concourse import bass_utils

    if "nc" not in _CACHE:
        _CACHE["nc"] = _build()
    nc = _CACHE["nc"]
    res = bass_utils.run_bass_kernel_spmd(
        nc, in_maps, core_ids=list(range(NCORES)), trace=trace,
    )
    LAST_EXEC_NS = getattr(res, "exec_time_ns", None)
    LAST_RESULTS = res
    return res.results


def kernel(x, Wq, bq, Wk, bk, Wv, bv, Wo, bo, attention_mask=None, _trace=False):
    """Full inputs in, full output out. attention_mask is the reference's
    causal mask; causality is hardcoded in the kernel."""
    in_maps = _prep_inputs(x, Wq, bq, Wk, bk, Wv, bv, Wo)
    results = _run(in_maps, trace=_trace)
    acc = np.zeros((BS, DM), dtype=np.float32)
    for c in range(NCORES):
        acc += results[c]["out"].astype(np.float32)
    acc += np.asarray(bo, np.float32)[None, :]
    return acc.reshape(B, S, DM)



# revision 13
# speedup vs baseline: 1.4065x; 1.4065x over previous
"""Multi-head attention (B=2, S=2048, DM=1024, H=16, DH=64, causal) on 8 TRN2 cores.

Sharding: tensor-parallel over heads. Core c owns heads {2c, 2c+1} = q/k/v dims
[128c, 128c+128). Each core computes its QKV projections, causal attention for
its 2 heads (both batches), and a partial output projection (row-parallel over
Wo). Host unshards by summing the 8 partials and adding bo (the TP all-reduce).

Design (v2 — fully pipelined single pass):
  - xT DMA'd in 8 seq-chunks so QKV compute starts after ~1MB lands.
  - QKV chunks interleaved with attention blocks: block (b,qb) is emitted as
    soon as chunks 0..b*4+qb are in SBUF, so ScalarE exp work starts early.
  - Scores: 2 heads row-packed on the PE (K=64 each, concurrent), f32 PSUM
    (128 keys, 1024 = 2x512 queries).
  - Softmax denominator for h0 FUSED into AV: V2 layout (128 seq, 130) =
    [Vh0 d0-63 | ones | Vh1 d0-63 | pad]; AV h0 lhsT=V2[0:65] -> ctxA[0:65]
    (sum0 at row 64). h1: AV lhsT=V2[65:129] -> ctxB[64:128] plus a 1-col
    ones matmul (lhsT=V2[64:65]) accumulating sum1 into ctxB[0:1] (matmul
    out base partition must be 0/32/64; engines cannot shift partitions).
  - Reciprocal via DVE reciprocal_approx_fast (~5x faster than reciprocal),
    in place at partitions 64 (sum0) / 0 (sum1).
  - rec broadcast across partitions via two K=1 PE matmuls (GpSimd
    partition_broadcast reads the wrong partition on HW; Pool can't see PSUM).
  - Diagonal causal mask via GpSimd affine_select on the exp tile (not DVE).
  - Tail (stash/recip/bcast/mult/outproj) software-pipelined into the NEXT
    attention block's emission so no engine drains.

Causality hardcoded (the reference's attention_mask is always triu causal).
"""

import os
import sys

import numpy as np

try:
    import concourse  # noqa: F401
except ImportError:
    sys.path.insert(0, "/opt/trn_rl_repo")

import ml_dtypes

BF16 = ml_dtypes.bfloat16

B, S, DM = 2, 2048, 1024
H, DH = 16, 64
NCORES = 8
CPC = DM // NCORES  # 128 q/k/v dims per core (2 heads)
BS = B * S  # 4096
Q_W = 512  # query-block width
N_CH = BS // Q_W  # 8 seq chunks
KT_FEAT = DM // 128  # 8 contraction tiles for QKV
NQB = S // Q_W  # 4 query blocks per batch

_CACHE = {}
LAST_EXEC_NS = None
LAST_RESULTS = None


def _build(repeat=1):
    # compat fallbacks for sim/HW divergence bisection (env: BENCH_COMPAT)
    compat = set(os.environ.get("BENCH_COMPAT", "").split(",")) - {""}
    C_MASK = "mask" in compat    # diag mask: DVE cmask mult, not affine_select
    C_EXP = "exp" in compat      # off>0 exp: two activations, not strided AP
    C_SUM = "sum" in compat      # sums_h1: separate PSUM tile (own bank)
    C_RECIP = "recip" in compat  # recip: copy to SBUF + exact reciprocal
    import concourse.mybir as mybir
    from concourse import bacc
    from concourse import tile
    from concourse.masks import make_identity, make_upper_triangular

    f32 = mybir.dt.float32
    f16 = mybir.dt.float16
    bf16 = mybir.dt.bfloat16
    Exp = mybir.ActivationFunctionType.Exp
    Ident = mybir.ActivationFunctionType.Identity
    is_ge = mybir.AluOpType.is_ge

    nc = bacc.Bacc(
        "TRN2",
        target_bir_lowering=False,
        debug=False,
        enable_asserts=False,
        num_devices=NCORES,
    )

    xT = nc.dram_tensor("xT", (DM, BS), bf16, kind="ExternalInput").ap()
    wq = nc.dram_tensor("wq", (DM, CPC), bf16, kind="ExternalInput").ap()
    wk = nc.dram_tensor("wk", (DM, CPC), bf16, kind="ExternalInput").ap()
    wv = nc.dram_tensor("wv", (DM, CPC), bf16, kind="ExternalInput").ap()
    wo = nc.dram_tensor("wo", (CPC, DM), bf16, kind="ExternalInput").ap()
    bq = nc.dram_tensor("bq", (CPC, 1), f32, kind="ExternalInput").ap()
    bk = nc.dram_tensor("bk", (CPC, 1), f32, kind="ExternalInput").ap()
    bv = nc.dram_tensor("bv", (CPC, 1), f32, kind="ExternalInput").ap()
    out = nc.dram_tensor("out", (BS, DM), f16, kind="ExternalOutput").ap()

    with tile.TileContext(nc) as tc:
      with tc.tile_pool(name="consts", bufs=1) as consts, \
           tc.tile_pool(name="sb", bufs=2) as sb, \
           tc.tile_pool(name="psp", bufs=1, space="PSUM") as psp:

        def body():
            # ---- persistent tiles ------------------------------------------
            wq_sb = consts.tile((128, KT_FEAT, CPC), bf16, name="wq_sb")
            wk_sb = consts.tile((128, KT_FEAT, CPC), bf16, name="wk_sb")
            wv_sb = consts.tile((128, KT_FEAT, CPC), bf16, name="wv_sb")
            wo_sb = consts.tile((CPC, DM), bf16, name="wo_sb")
            bq_sb = consts.tile((CPC, 1), f32, name="bq_sb")
            bk_sb = consts.tile((CPC, 1), f32, name="bk_sb")
            bv_sb = consts.tile((CPC, 1), f32, name="bv_sb")
            nc.sync.dma_start(wq_sb[:], wq.rearrange("(t p) m -> p t m", p=128))
            nc.sync.dma_start(wk_sb[:], wk.rearrange("(t p) m -> p t m", p=128))
            nc.sync.dma_start(wv_sb[:], wv.rearrange("(t p) m -> p t m", p=128))
            nc.sync.dma_start(wo_sb[:], wo)
            nc.sync.dma_start(bq_sb[:], bq)
            nc.sync.dma_start(bk_sb[:], bk)
            nc.sync.dma_start(bv_sb[:], bv)

            ident_sb = consts.tile((128, 128), bf16, name="ident_sb")
            make_identity(nc, ident_sb[:])
            ones_bf = consts.tile((128, 64), bf16, name="ones_bf")
            nc.vector.memset(ones_bf[:], 1.0)
            if C_MASK:
                cmask = consts.tile((128, 128), bf16, name="cmask")
                make_upper_triangular(nc, cmask[:], val=1.0, diag=True)
            CTX_BUFS = 3 if C_SUM else 2
            O_BUFS = 1 if C_SUM else 2

            QT_sb = consts.tile((128, BS), bf16, name="QT_sb")
            KT_sb = consts.tile((128, BS), bf16, name="KT_sb")
            # V2: (seq 128, 32 k-tiles, 130) = [Vh0 d0-63 | ones | Vh1 d0-63 | pad]
            V2_sb = consts.tile((128, BS // 128, 130), bf16, name="V2_sb")
            nc.vector.memset(V2_sb[:, :, 64:65], 1.0)

            # xT in 8 per-chunk tiles so QKV(ch) only waits its own slab
            xT3 = xT.rearrange("(t p) q -> p t q", p=128)
            xts = []
            for ch in range(N_CH):
                xt = consts.tile((128, KT_FEAT, Q_W), bf16, name=f"xt{ch}")
                nc.sync.dma_start(xt[:], xT3[:, :, ch * Q_W:(ch + 1) * Q_W])
                xts.append(xt)

            # ---- QKV chunk -------------------------------------------------
            def emit_qkv(ch):
                c0 = ch * Q_W
                vt = sb.tile((128, Q_W), bf16, name=f"vt{ch}", tag="vt",
                             bufs=2)
                for pname, w_sb, b_sb in (
                    ("q", wq_sb, bq_sb),
                    ("k", wk_sb, bk_sb),
                    ("v", wv_sb, bv_sb),
                ):
                    ps_p = psp.tile((128, Q_W), f32, name=f"ps_{pname}{ch}",
                                    tag="s", bufs=2)
                    for t in range(KT_FEAT):
                        nc.tensor.matmul(
                            ps_p[:],
                            lhsT=w_sb[:, t, :],
                            rhs=xts[ch][:, t, :],
                            start=(t == 0),
                            stop=(t == KT_FEAT - 1),
                        )
                    if pname == "q":
                        nc.scalar.activation(QT_sb[:, c0:c0 + Q_W], ps_p[:],
                                             Ident, bias=b_sb[:])
                    elif pname == "k":
                        nc.scalar.activation(KT_sb[:, c0:c0 + Q_W], ps_p[:],
                                             Ident, bias=b_sb[:])
                    else:
                        nc.scalar.activation(vt[:], ps_p[:], Ident,
                                             bias=b_sb[:])

                # V natural layout via PE transpose; strided copy drops the
                # transposed cols into [0:64] and [65:129] around the ones col
                for sub in range(4):
                    ps_t = psp.tile((128, 128), bf16, name=f"ps_t{ch}_{sub}",
                                    tag="o", bufs=O_BUFS)
                    nc.tensor.transpose(
                        ps_t[:],
                        vt[:, sub * 128:(sub + 1) * 128],
                        ident_sb[:],
                    )
                    ti = ch * 4 + sub
                    dst = V2_sb[:, ti, :].rearrange(
                        "p (a b) -> p a b", b=65)[:, :, 0:64]
                    src = ps_t[:].rearrange("p (a b) -> p a b", b=64)
                    nc.vector.tensor_copy(dst, src)

            # ---- attention block (b, qb) with pipelined tail ---------------
            def make_tail(b, qb, ctxA, ctxB, ctxS):
                g0 = b * S + qb * Q_W
                hold = {}

                def stash():
                    cu = sb.tile((128, Q_W), bf16, name=f"cu{b}_{qb}",
                                 tag="cu", bufs=2)
                    nc.vector.tensor_copy(cu[0:64, :], ctxA[0:64, :])
                    nc.vector.tensor_copy(cu[64:128, :], ctxB[64:128, :])
                    hold["cu"] = cu

                def recips():
                    st = sb.tile((128, Q_W), f32, name=f"st{b}_{qb}",
                                 tag="st", bufs=2)
                    # custom-DVE ops (reciprocal_approx_*) silently misread
                    # on HW when the AP base partition != 0, and can't read
                    # PSUM: bounce sums rows into a memset SBUF tile and do
                    # ONE full-tile recip at base 0 (cost is free-size-driven)
                    sti = sb.tile((128, Q_W), f32, name=f"sti{b}_{qb}",
                                  tag="sti", bufs=2)
                    nc.vector.memset(sti[:], 1.0)
                    nc.vector.tensor_copy(sti[64:65, :], ctxA[64:65, :])
                    nc.vector.tensor_copy(sti[0:1, :], ctxS[0:1, :])
                    if C_RECIP:
                        nc.vector.reciprocal(st[64:65, :], sti[64:65, :])
                        nc.vector.reciprocal(st[0:1, :], sti[0:1, :])
                    else:
                        nc.vector.reciprocal_approx_fast(st[:], sti[:])
                    stb = sb.tile((128, Q_W), bf16, name=f"stb{b}_{qb}",
                                  tag="stb", bufs=2)
                    if C_RECIP:
                        nc.vector.tensor_copy(stb[64:65, :], st[64:65, :])
                        nc.vector.tensor_copy(stb[0:1, :], st[0:1, :])
                    else:
                        nc.vector.tensor_copy(stb[:], st[:])
                    hold["stb"] = stb

                def bcast_mult():
                    stb = hold["stb"]
                    ps_bc = psp.tile((128, Q_W), f32, name=f"ps_bc{b}_{qb}",
                                     tag="o", bufs=O_BUFS)
                    nc.tensor.matmul(ps_bc[0:64, :],
                                     lhsT=ones_bf[64:65, 0:64],
                                     rhs=stb[64:65, :])
                    nc.tensor.matmul(ps_bc[64:128, :],
                                     lhsT=ones_bf[0:1, 0:64],
                                     rhs=stb[0:1, :])
                    cx = sb.tile((128, Q_W), bf16, name=f"cx{b}_{qb}",
                                 tag="cx", bufs=2)
                    nc.vector.tensor_mul(cx[:], hold["cu"][:], ps_bc[:])
                    hold["cx"] = cx

                def outproj(k):
                    cx = hold["cx"]
                    for sub in (2 * k, 2 * k + 1):
                        o_sb = sb.tile((128, DM), f16,
                                       name=f"o{b}_{qb}_{sub}",
                                       tag="ob", bufs=3)
                        for nn in range(2):
                            ps_o = psp.tile((128, 512), f32,
                                            name=f"ps_o{b}_{qb}_{sub}_{nn}",
                                            tag="o", bufs=O_BUFS)
                            nc.tensor.matmul(
                                ps_o[:],
                                lhsT=cx[:, sub * 128:(sub + 1) * 128],
                                rhs=wo_sb[:, nn * 512:(nn + 1) * 512],
                            )
                            nc.vector.tensor_copy(
                                o_sb[:, nn * 512:(nn + 1) * 512], ps_o[:])
                        r0 = g0 + sub * 128
                        nc.sync.dma_start(out[r0:r0 + 128, :], o_sb[:])

                return [stash, recips, bcast_mult,
                        lambda: outproj(0), lambda: outproj(1)]

            def emit_attn(b, qb, pend):
                qb0 = qb * Q_W
                g0 = b * S + qb0
                n_t = (qb0 + Q_W) // 128  # causal: k-tiles needed
                ctxA = psp.tile((128, Q_W), f32, name=f"ctxA{b}_{qb}",
                                tag="ctx", bufs=CTX_BUFS)
                ctxB = psp.tile((128, Q_W), f32, name=f"ctxB{b}_{qb}",
                                tag="ctx", bufs=CTX_BUFS)
                ctxS = (psp.tile((128, Q_W), f32, name=f"ctxS{b}_{qb}",
                                 tag="ctx", bufs=CTX_BUFS) if C_SUM else ctxB)

                avq = []  # delayed AV args: (exp_sb, t, off, w)

                def emit_av(exp_sb, t, off, w):
                    first = t == 0
                    last = t == n_t - 1
                    kti = (b * S + 128 * t) // 128
                    nc.tensor.matmul(
                        ctxA[0:65, off:Q_W],
                        lhsT=V2_sb[:, kti, 0:65],
                        rhs=exp_sb[:, 0:w],
                        start=first,
                        stop=last,
                    )
                    nc.tensor.matmul(
                        ctxB[64:128, off:Q_W],
                        lhsT=V2_sb[:, kti, 65:129],
                        rhs=exp_sb[:, 512:512 + w],
                        start=first,
                        stop=last,
                        skip_group_check=True,
                    )
                    nc.tensor.matmul(
                        ctxS[0:1, off:Q_W],
                        lhsT=V2_sb[:, kti, 64:65],
                        rhs=exp_sb[:, 512:512 + w],
                        start=first,
                        stop=last,
                        skip_group_check=True,
                    )

                for t in range(n_t):
                    k0 = 128 * t
                    off = max(0, k0 - qb0)
                    w = Q_W - off
                    diag = k0 >= qb0
                    ps_s = psp.tile((128, 1024), f32,
                                    name=f"ps_s{b}_{qb}_{t}",
                                    tag="s", bufs=2)
                    exp_sb = sb.tile((128, 1024), bf16,
                                     name=f"exp{b}_{qb}_{t}",
                                     tag="exp", bufs=4)
                    for h in range(2):
                        nc.tensor.matmul(
                            ps_s[:, h * 512:h * 512 + w],
                            lhsT=KT_sb[h * 64:(h + 1) * 64,
                                       b * S + k0:b * S + k0 + 128],
                            rhs=QT_sb[h * 64:(h + 1) * 64,
                                      g0 + off:g0 + Q_W],
                            start=True,
                            stop=True,
                            tile_position=(h * 64, 0),
                            skip_group_check=True,
                        )
                    if t == 0 and pend:
                        pend[0]()  # stash(prev): frees ctx ring slots
                    if off == 0:
                        nc.scalar.activation(exp_sb[:, :1024], ps_s[:, :1024],
                                             Exp, scale=0.125)
                    elif C_EXP:
                        nc.scalar.activation(exp_sb[:, 0:w], ps_s[:, 0:w],
                                             Exp, scale=0.125)
                        nc.scalar.activation(exp_sb[:, 512:512 + w],
                                             ps_s[:, 512:512 + w],
                                             Exp, scale=0.125)
                    else:
                        s3 = ps_s[:].rearrange("p (h q) -> p h q",
                                               h=2)[:, :, 0:w]
                        e3 = exp_sb[:].rearrange("p (h q) -> p h q",
                                                 h=2)[:, :, 0:w]
                        nc.scalar.activation(e3, s3, Exp, scale=0.125)
                    if t == 0 and pend:
                        pend[1]()  # recips(prev): last ctx(prev) readers
                    if diag and C_MASK:
                        for h in range(2):
                            sb0 = h * 512
                            nc.vector.tensor_mul(
                                exp_sb[:, sb0:sb0 + 128],
                                exp_sb[:, sb0:sb0 + 128],
                                cmask[:],
                            )
                    elif diag:
                        em = exp_sb[:].rearrange("p (h q) -> p h q",
                                                 h=2)[:, :, 0:128]
                        nc.gpsimd.affine_select(
                            em, em,
                            pattern=[[0, 2], [1, 128]],
                            compare_op=is_ge,
                            fill=0.0,
                            base=0,
                            channel_multiplier=-1,
                        )
                    if t >= 1:
                        emit_av(*avq[t - 1])
                    avq.append((exp_sb, t, off, w))
                    if t == 1 and pend:
                        pend[2]()  # bcast+mult(prev)
                    if t == 2 and pend:
                        pend[3]()  # outproj(prev) subs 0-1
                    if t == 3 and pend:
                        pend[4]()  # outproj(prev) subs 2-3
                emit_av(*avq[n_t - 1])
                return make_tail(b, qb, ctxA, ctxB, ctxS)

            # ---- interleaved emission --------------------------------------
            done_ch = 0
            pend = []
            for b in range(B):
                for qb in range(NQB):
                    need = b * NQB + qb + 1
                    while done_ch < need:
                        emit_qkv(done_ch)
                        done_ch += 1
                    pend = emit_attn(b, qb, pend)
            while done_ch < N_CH:
                emit_qkv(done_ch)
                done_ch += 1
            for step in pend:
                step()

        if repeat == 1:
            body()
        else:
            with tc.For_i(0, repeat, 1):
                body()

    nc.compile()
    return nc


def _prep_inputs(x, Wq, bq, Wk, bk, Wv, bv, Wo):
    """Build the 8 per-core input maps (host-side sharding)."""
    x = np.asarray(x, dtype=np.float32)
    xT = np.ascontiguousarray(x.reshape(BS, DM).T).astype(BF16)
    in_maps = []
    for c in range(NCORES):
        sl = slice(c * CPC, (c + 1) * CPC)
        in_maps.append({
            "xT": xT,
            "wq": np.ascontiguousarray(np.asarray(Wq, np.float32)[sl, :].T).astype(BF16),
            "wk": np.ascontiguousarray(np.asarray(Wk, np.float32)[sl, :].T).astype(BF16),
            "wv": np.ascontiguousarray(np.asarray(Wv, np.float32)[sl, :].T).astype(BF16),
            "wo": np.ascontiguousarray(np.asarray(Wo, np.float32)[:, sl].T).astype(BF16),
            "bq": np.asarray(bq, np.float32)[sl].reshape(CPC, 1).copy(),
            "bk": np.asarray(bk, np.float32)[sl].reshape(CPC, 1).copy(),
            "bv": np.asarray(bv, np.float32)[sl].reshape(CPC, 1).copy(),
        })
    return in_maps


def _run(in_maps, trace=False):
    global LAST_EXEC_NS, LAST_RESULTS
    from concourse import bass_utils

    if "nc" not in _CACHE:
        _CACHE["nc"] = _build()
    nc = _CACHE["nc"]
    res = bass_utils.run_bass_kernel_spmd(
        nc, in_maps, core_ids=list(range(NCORES)), trace=trace,
    )
    LAST_EXEC_NS = getattr(res, "exec_time_ns", None)
    LAST_RESULTS = res
    return res.results


def kernel(x, Wq, bq, Wk, bk, Wv, bv, Wo, bo, attention_mask=None, _trace=False):
    """Full inputs in, full output out. attention_mask is the reference's
    causal mask; causality is hardcoded in the kernel."""
    in_maps = _prep_inputs(x, Wq, bq, Wk, bk, Wv, bv, Wo)
    results = _run(in_maps, trace=_trace)
    acc = np.zeros((BS, DM), dtype=np.float32)
    for c in range(NCORES):
        acc += results[c]["out"].astype(np.float32)
    acc += np.asarray(bo, np.float32)[None, :]
    return acc.reshape(B, S, DM)
